# revision 6
# baseline (speedup 1.0000x reference)
"""Trainium2 Bass kernel: Kuramoto GNN message passing on 8 NeuronCores.

accel[u] = (power[u] - gamma[u]*dphase[u] + S[u]) / mass[u]
  S[u] = sum over directed edges (u <- v) of K_e * sin(phase[v] - phase[u])

Directed edges (both directions of every undirected edge) are sharded by dst
range: core i owns dst in [i*62500, (i+1)*62500).  Host work is indexing and
layout only: per core, edges are bucketed by dst and laid out in a dense
degree-padded stream.  Nodes are ranked by degree (descending) so that each
block of 128 consecutive ranks has a near-uniform padded segment length L
(multiple of 8, max over all cores for SPMD uniformity).  The stream holds,
per node, its edges' (delta = phase[src]-phase[dst], K) padded with zeros to
L slots.  The device computes sin (ScalarE), the K*sin product (VectorE), the
per-node segment sums via strided tensor_reduce (VectorE), and the final
elementwise combine with reciprocal (VectorE).  No scatter, no collectives:
output slices are disjoint per core and concatenated on the host.
"""
import numpy as np
from contextlib import ExitStack

import concourse.bass as bass
import concourse.bacc as bacc
import concourse.mybir as mybir
from concourse.bass_utils import run_bass_kernel_spmd

N = 500_000
NCORES = 8
RS = N // NCORES            # 62500 dst nodes per core
BPC = (RS + 127) // 128     # 489 rank-blocks of 128 nodes
RT = BPC                    # columns of the [128, RT] node layout
NPAD = BPC * 128            # 62592 ranks incl. dummy tail
WMAX = 4096                 # max piece free-width (f32 per partition)
NB = 3                      # pipeline ring depth
PI = np.float32(np.pi)
TWO_PI = np.float32(2.0 * np.pi)


def _schedule(Lb):
    """Group consecutive equal-L blocks into pieces of width <= WMAX.

    Returns list of (W0, c0, R, L): piece reads stream cols [W0, W0+R*L),
    reducing into S cols [c0, c0+R).
    """
    cb = np.concatenate([[0], np.cumsum(Lb)]).astype(np.int64)
    pieces = []
    c = 0
    while c < len(Lb):
        L = int(Lb[c])
        e = c
        while e < len(Lb) and Lb[e] == L:
            e += 1
        rmax = max(1, WMAX // L)
        while c < e:
            R = min(rmax, e - c)
            pieces.append((int(cb[c]), c, R, L))
            c += R
    return pieces, int(cb[-1])


def _build(pieces, TOTW):
    NP = len(pieces)
    WBUF = max(R * L for (_, _, R, L) in pieces)
    nc = bacc.Bacc("TRN2", debug=False)
    ph_h = nc.dram_tensor("ph", [128, TOTW], mybir.dt.float32, kind="ExternalInput")
    w_h = nc.dram_tensor("w", [128, TOTW], mybir.dt.float32, kind="ExternalInput")
    nodes_h = nc.dram_tensor("nodes", [4, 128, RT], mybir.dt.float32, kind="ExternalInput")
    out_h = nc.dram_tensor("out", [128, RT], mybir.dt.float32, kind="ExternalOutput")

    with (
        nc.Block() as block,
        nc.sbuf_tensor("phb", [128, NB * WBUF], mybir.dt.float32) as phb,
        nc.sbuf_tensor("wb", [128, NB * WBUF], mybir.dt.float32) as wb,
        nc.sbuf_tensor("vb", [128, NB * WBUF], mybir.dt.float32) as vb,
        nc.sbuf_tensor("scb", [128, RT], mybir.dt.float32) as scb,
        nc.sbuf_tensor("ndb", [128, 4 * RT], mybir.dt.float32) as ndb,
        nc.sbuf_tensor("tb", [128, RT], mybir.dt.float32) as tb,
        nc.sbuf_tensor("rb", [128, RT], mybir.dt.float32) as rb,
        nc.sbuf_tensor("fin", [128, RT], mybir.dt.float32) as fin,
        nc.semaphore("ion") as ion,
        nc.semaphore("acs") as acs,
        nc.semaphore("dvs") as dvs,
        nc.semaphore("od") as od,
        nc.semaphore("mm") as mm,
        nc.semaphore("fs") as fs,
        ExitStack() as stack,
    ):
        # One DMA-completion semaphore per ring slot per stream: only one
        # in-flight DMA increments a given semaphore at a time (a dma_start's
        # +16 arrives as interleavable partial bumps, so concurrent DMAs may
        # not share a semaphore the consumer waits at intermediate values).
        iod = [stack.enter_context(nc.semaphore(f"iod{k}")) for k in range(NB)]
        iow = [stack.enter_context(nc.semaphore(f"iow{k}")) for k in range(NB)]
        def PH(i):
            return phb[:, (i % NB) * WBUF:(i % NB) * WBUF + pieces[i][2] * pieces[i][3]]

        def WT(i):
            return wb[:, (i % NB) * WBUF:(i % NB) * WBUF + pieces[i][2] * pieces[i][3]]

        def VB(i):
            return vb[:, (i % NB) * WBUF:(i % NB) * WBUF + pieces[i][2] * pieces[i][3]]

        @block.sync
        def _(sp):
            for f in range(4):
                sp.dma_start(ndb[:, f * RT:(f + 1) * RT], nodes_h[f]).then_inc(ion, 16)
            for i, (W0, c0, R, L) in enumerate(pieces):
                if i >= NB:
                    sp.wait_ge(acs, i - NB + 1)      # delta buf consumed by sin
                sp.dma_start(PH(i), ph_h[:, W0:W0 + R * L]).then_inc(iod[i % NB], 16)
                if i >= NB:
                    sp.wait_ge(dvs, i - NB + 1)      # w buf consumed by mul
                sp.dma_start(WT(i), w_h[:, W0:W0 + R * L]).then_inc(iow[i % NB], 16)
            sp.wait_ge(dvs, NP + 1)                  # final combine done
            sp.dma_start(out_h[:], fin[:]).then_inc(od, 16)
            sp.wait_ge(od, 16)

        @block.scalar
        def _(se):
            for i in range(NP):
                se.wait_ge(iod[i % NB], 16 * (i // NB + 1))
                if i >= NB:
                    se.wait_ge(dvs, i - NB + 1)      # vb slot consumed by reduce
                se.activation(VB(i), PH(i), mybir.ActivationFunctionType.Sin
                              ).then_inc(acs, 1)

        @block.vector
        def _(ve):
            for i, (W0, c0, R, L) in enumerate(pieces):
                ve.wait_ge(acs, i + 1)
                ve.wait_ge(iow[i % NB], 16 * (i // NB + 1))
                # Engine instructions pipeline; the self-semaphore orders the
                # reduce's read of vb after the in-place mul fully retires.
                ve.tensor_mul(VB(i), VB(i), WT(i)).then_inc(mm, 1)
                v3 = VB(i).rearrange("p (r l) -> p r l", l=L)
                ve.wait_ge(mm, i + 1)
                ve.tensor_reduce(scb[:, c0:c0 + R], v3, axis=mybir.AxisListType.X,
                                 op=mybir.AluOpType.add).then_inc(dvs, 1)
            ve.wait_ge(ion, 64)
            ve.wait_ge(dvs, NP)
            po = ndb[:, 0:RT]
            ga = ndb[:, RT:2 * RT]
            dp = ndb[:, 2 * RT:3 * RT]
            ma = ndb[:, 3 * RT:4 * RT]
            ve.reciprocal(rb[:], ma[:]).then_inc(fs, 1)
            ve.tensor_mul(tb[:], ga[:], dp[:]).then_inc(fs, 1)
            ve.wait_ge(fs, 2)
            ve.tensor_sub(tb[:], po[:], tb[:]).then_inc(fs, 1)
            ve.wait_ge(fs, 3)
            ve.tensor_add(tb[:], tb[:], scb[:]).then_inc(fs, 1)
            ve.wait_ge(fs, 4)
            ve.tensor_mul(fin[:], tb[:], rb[:]).then_inc(dvs, 1)

    nc.compile()
    nc.finalize()
    return nc


_CACHE = {}


def _prep(phase, K, edge_index):
    """Host layout: dst-bucketed degree-padded streams + node permutation."""
    ei = np.asarray(edge_index)
    row = ei[0].astype(np.int32)
    col = ei[1].astype(np.int32)
    dst = np.concatenate([row, col])
    src = np.concatenate([col, row])
    w = np.concatenate([K, K]).astype(np.float32)

    order = np.argsort(dst, kind="stable")
    dsts = dst[order]
    srcs = src[order]
    ws = w[order]
    deg = np.bincount(dsts, minlength=N).astype(np.int32)
    starts = np.concatenate([[0], np.cumsum(deg)]).astype(np.int64)
    occ = (np.arange(dsts.size, dtype=np.int64) - starts[dsts]).astype(np.int32)

    delta = phase[srcs] - phase[dsts]
    delta = np.mod(delta + PI, TWO_PI) - PI

    deg2 = deg.reshape(NCORES, RS)
    rank_order = np.argsort(-deg2, axis=1, kind="stable").astype(np.int32)
    degsorted = np.take_along_axis(deg2, rank_order, axis=1)
    dpad = np.zeros((NCORES, NPAD), np.int32)
    dpad[:, :RS] = degsorted
    Lb = dpad.reshape(NCORES, BPC, 128).max(axis=2).max(axis=0)
    Lb = np.maximum(((Lb + 7) // 8) * 8, 8).astype(np.int64)

    pieces, TOTW = _schedule(Lb)
    cb = np.concatenate([[0], np.cumsum(Lb)]).astype(np.int64)

    rank_of = np.empty((NCORES, RS), np.int32)
    np.put_along_axis(rank_of, rank_order,
                      np.broadcast_to(np.arange(RS, dtype=np.int32), (NCORES, RS)),
                      axis=1)

    core_e = dsts // RS
    dloc_e = dsts - core_e * RS
    rank_e = rank_of[core_e, dloc_e].astype(np.int64)
    p_e = rank_e % 128
    c_e = rank_e // 128
    flat = (core_e.astype(np.int64) * 128 + p_e) * TOTW + cb[c_e] + occ

    ph_str = np.zeros(NCORES * 128 * TOTW, np.float32)
    w_str = np.zeros(NCORES * 128 * TOTW, np.float32)
    ph_str[flat] = delta
    w_str[flat] = ws
    ph_str = ph_str.reshape(NCORES, 128, TOTW)
    w_str = w_str.reshape(NCORES, 128, TOTW)
    return pieces, TOTW, ph_str, w_str, rank_order


def kernel(phase, dphase, power, mass, gamma, K, edge_index):
    phase = np.asarray(phase, np.float32)
    dphase = np.asarray(dphase, np.float32)
    power = np.asarray(power, np.float32)
    mass = np.asarray(mass, np.float32)
    gamma = np.asarray(gamma, np.float32)
    K = np.asarray(K, np.float32)

    pieces, TOTW, ph_str, w_str, rank_order = _prep(phase, K, edge_index)
    key = (TOTW, tuple(pieces))
    if key not in _CACHE:
        _CACHE[key] = _build(pieces, TOTW)
    nc = _CACHE[key]

    in_maps = []
    for ci in range(NCORES):
        nodes = np.zeros((4, NPAD), np.float32)
        nodes[3, RS:] = 1.0                      # dummy-rank mass
        ro = rank_order[ci]
        sl = slice(ci * RS, (ci + 1) * RS)
        nodes[0, :RS] = power[sl][ro]
        nodes[1, :RS] = gamma[sl][ro]
        nodes[2, :RS] = dphase[sl][ro]
        nodes[3, :RS] = mass[sl][ro]
        # rank r = 128*c + p  ->  [128, RT] at (p, c)
        nodes4 = np.ascontiguousarray(
            nodes.reshape(4, RT, 128).transpose(0, 2, 1))
        in_maps.append({"ph": ph_str[ci], "w": w_str[ci], "nodes": nodes4})

    res = run_bass_kernel_spmd(nc, in_maps, core_ids=list(range(NCORES)))
    out = np.empty(N, np.float32)
    for ci in range(NCORES):
        o = res.results[ci]["out"]               # [128, RT], rank = 128*c + p
        by_rank = o.T.reshape(-1)[:RS]
        out[ci * RS + rank_order[ci]] = by_rank
    return out


# revision 28
# speedup vs baseline: 1.1131x; 1.1131x over previous
"""Trainium2 Bass kernel: Kuramoto GNN message passing on 8 NeuronCores.

accel[u] = (power[u] - gamma[u]*dphase[u] + S[u]) / mass[u]
  S[u] = sum over directed edges (u <- v) of K_e * sin(phase[v] - phase[u])

Directed edges (both directions of every undirected edge) are sharded by dst
range: core i owns dst in [i*62500, (i+1)*62500).  Host work is indexing and
layout only: per core, edges are bucketed by dst and laid out in a dense
degree-padded stream.  Nodes are ranked by degree (descending) so that each
block of 128 consecutive ranks has a near-uniform padded segment length L
(multiple of 8, max over all cores for SPMD uniformity).  The stream holds,
per node, its edges' (delta = phase[src]-phase[dst], K) padded with zeros to
L slots.  The device computes sin (ScalarE), the K*sin product (VectorE), the
per-node segment sums via strided tensor_reduce (VectorE), and the final
elementwise combine with reciprocal (VectorE).  No scatter, no collectives:
output slices are disjoint per core and concatenated on the host.
"""
import numpy as np
from contextlib import ExitStack

import concourse.bass as bass
import concourse.bacc as bacc
import concourse.mybir as mybir
from concourse.bass_utils import run_bass_kernel_spmd

N = 500_000
NCORES = 8
RS = N // NCORES            # 62500 dst nodes per core
BPC = (RS + 127) // 128     # 489 rank-blocks of 128 nodes
RT = BPC                    # columns of the [128, RT] node layout
NPAD = BPC * 128            # 62592 ranks incl. dummy tail
WMAX = 2560                 # max piece free-width (f32 per partition)
NB = 5                      # pipeline ring depth
MINB = 12                   # min blocks per class run (1 = no coalescing)
QL = 2                      # quantization of per-block padded length L
TAPER = 0                   # split the last piece into 2^TAPER shrinking chunks
PI = np.float32(np.pi)
TWO_PI = np.float32(2.0 * np.pi)


def _schedule(Lb):
    """Group consecutive equal-L blocks into pieces of width <= WMAX.

    Pieces are ordered smallest-first then descending by width so the
    pipeline fills fast and the post-last-DMA tail (sin+mul+reduce of the
    final piece) is short.  Returns (pieces, TOTW, colbase) where pieces is
    a list of (W0, c0, R, L): the piece reads stream cols [W0, W0+R*L) and
    reduces into S cols [c0, c0+R); colbase[c] is the stream column where
    rank-block c's slots start.
    """
    raw = []
    c = 0
    while c < len(Lb):
        L = int(Lb[c])
        e = c
        while e < len(Lb) and Lb[e] == L:
            e += 1
        rmax = max(1, WMAX // L)
        while c < e:
            R = min(rmax, e - c)
            raw.append((c, R, L))
            c += R
    raw.sort(key=lambda p: p[1] * p[2], reverse=True)
    if len(raw) >= 2:
        # second-smallest first (fast pipeline fill), smallest last (short
        # post-last-DMA tail), big pieces in the middle.
        raw = raw[-2:-1] + raw[:-2] + raw[-1:]
    for _ in range(TAPER):
        c0, R, L = raw[-1]
        if R < 4:
            break
        h = R // 2
        raw[-1:] = [(c0, R - h, L), (c0 + R - h, h, L)]
    pieces = []
    colbase = np.zeros(len(Lb), np.int64)
    W0 = 0
    for (c0, R, L) in raw:
        pieces.append((W0, c0, R, L))
        colbase[c0:c0 + R] = W0 + np.arange(R, dtype=np.int64) * L
        W0 += R * L
    return pieces, W0, colbase


def _build(pieces, TOTW):
    NP = len(pieces)
    WBUF = max(R * L for (_, _, R, L) in pieces)
    nc = bacc.Bacc("TRN2", debug=False)
    ph_h = nc.dram_tensor("ph", [128, TOTW], mybir.dt.float32, kind="ExternalInput")
    w_h = nc.dram_tensor("w", [128, TOTW], mybir.dt.float32, kind="ExternalInput")
    nodes_h = nc.dram_tensor("nodes", [4, 128, RT], mybir.dt.float32, kind="ExternalInput")
    out_h = nc.dram_tensor("out", [128, RT], mybir.dt.float32, kind="ExternalOutput")

    with (
        nc.Block() as block,
        nc.sbuf_tensor("phb", [128, NB * WBUF], mybir.dt.float32) as phb,
        nc.sbuf_tensor("wb", [128, NB * WBUF], mybir.dt.float32) as wb,
        nc.sbuf_tensor("vb", [128, NB * WBUF], mybir.dt.float32) as vb,
        nc.sbuf_tensor("scb", [128, RT], mybir.dt.float32) as scb,
        nc.sbuf_tensor("ndb", [128, 4 * RT], mybir.dt.float32) as ndb,
        nc.sbuf_tensor("tb", [128, RT], mybir.dt.float32) as tb,
        nc.sbuf_tensor("rb", [128, RT], mybir.dt.float32) as rb,
        nc.sbuf_tensor("fin", [128, RT], mybir.dt.float32) as fin,
        nc.sbuf_tensor("scr", [128, 1], mybir.dt.float32) as scr,
        nc.semaphore("ion") as ion,
        nc.semaphore("acs") as acs,
        nc.semaphore("dvs") as dvs,
        nc.semaphore("od") as od,
        nc.semaphore("mm") as mm,
        nc.semaphore("fs") as fs,
        nc.semaphore("f2") as f2,
        nc.semaphore("fss") as fss,
        ExitStack() as stack,
    ):
        # One DMA-completion semaphore per ring slot per stream: only one
        # in-flight DMA increments a given semaphore at a time (a dma_start's
        # +16 arrives as interleavable partial bumps, so concurrent DMAs may
        # not share a semaphore the consumer waits at intermediate values).
        iod = [stack.enter_context(nc.semaphore(f"iod{k}")) for k in range(NB)]
        iow = [stack.enter_context(nc.semaphore(f"iow{k}")) for k in range(NB)]
        def PH(i):
            return phb[:, (i % NB) * WBUF:(i % NB) * WBUF + pieces[i][2] * pieces[i][3]]

        def WT(i):
            return wb[:, (i % NB) * WBUF:(i % NB) * WBUF + pieces[i][2] * pieces[i][3]]

        def VB(i):
            return vb[:, (i % NB) * WBUF:(i % NB) * WBUF + pieces[i][2] * pieces[i][3]]

        @block.sync
        def _(sp):
            LAG = NB + 1   # out-DMA for piece i-LAG interleaves with piece i
            sp.dma_start(ndb[:].rearrange("p (f c) -> p f c", f=4),
                         nodes_h[:].rearrange("f p c -> p f c")).then_inc(ion, 16)

            def out_dma(j):
                _, c0, R, _ = pieces[j]
                sp.wait_ge(fss, j + 1)
                with nc.allow_non_contiguous_dma(reason="R=1 out slice is 128x4B"):
                    sp.dma_start(out_h[:, c0:c0 + R],
                                 fin[:, c0:c0 + R]).then_inc(od, 16)

            for i, (W0, c0, R, L) in enumerate(pieces):
                if i >= NB:
                    sp.wait_ge(acs, i - NB + 2)      # delta buf consumed by sin
                sp.dma_start(PH(i), ph_h[:, W0:W0 + R * L]).then_inc(iod[i % NB], 16)
                if i >= NB:
                    sp.wait_ge(dvs, i - NB + 1)      # w buf consumed by mul
                sp.dma_start(WT(i), w_h[:, W0:W0 + R * L]).then_inc(iow[i % NB], 16)
                if i >= LAG:
                    out_dma(i - LAG)
            for j in range(max(0, NP - LAG), NP):
                out_dma(j)
            sp.wait_ge(od, 16 * NP)

        @block.scalar
        def _(se):
            # Dummy activation to front-load the activation-table loads.
            zero = nc.const_aps.tensor(0.0, (128, 1), mybir.dt.float32)
            se.activation(scr[:], zero, mybir.ActivationFunctionType.Sin
                          ).then_inc(acs, 1)
            for i in range(NP):
                se.wait_ge(iod[i % NB], 16 * (i // NB + 1))
                if i >= NB:
                    se.wait_ge(dvs, i - NB + 1)      # vb slot consumed by reduce
                se.activation(VB(i), PH(i), mybir.ActivationFunctionType.Sin
                              ).then_inc(acs, 1)

        @block.vector
        def _(ve):
            # Prologue while the first stream DMAs are in flight:
            # rb = 1/mass, tb = power - gamma*dphase.
            po = ndb[:, 0:RT]
            ga = ndb[:, RT:2 * RT]
            dp = ndb[:, 2 * RT:3 * RT]
            ma = ndb[:, 3 * RT:4 * RT]
            ve.wait_ge(ion, 16)
            ve.reciprocal(rb[:], ma[:]).then_inc(fs, 1)
            ve.tensor_mul(tb[:], ga[:], dp[:]).then_inc(fs, 1)
            ve.wait_ge(fs, 2)
            ve.tensor_sub(tb[:], po[:], tb[:]).then_inc(fs, 1)
            for i, (W0, c0, R, L) in enumerate(pieces):
                ve.wait_ge(acs, i + 2)
                ve.wait_ge(iow[i % NB], 16 * (i // NB + 1))
                # Engine instructions pipeline; self-semaphores order each
                # read of a same-engine write after the writer fully retires.
                ve.tensor_mul(VB(i), VB(i), WT(i)).then_inc(mm, 1)
                v3 = VB(i).rearrange("p (r l) -> p r l", l=L)
                ve.wait_ge(mm, i + 1)
                ve.tensor_reduce(scb[:, c0:c0 + R], v3, axis=mybir.AxisListType.X,
                                 op=mybir.AluOpType.add).then_inc(dvs, 1)
                # Streamed final combine for this piece's columns.
                ve.wait_ge(dvs, i + 1)
                ve.wait_ge(fs, 3)
                ve.tensor_add(fin[:, c0:c0 + R], tb[:, c0:c0 + R],
                              scb[:, c0:c0 + R]).then_inc(f2, 1)
                ve.wait_ge(f2, i + 1)
                ve.tensor_mul(fin[:, c0:c0 + R], fin[:, c0:c0 + R],
                              rb[:, c0:c0 + R]).then_inc(fss, 1)

    nc.compile()
    nc.finalize()
    return nc


_CACHE = {}


def _blocks(deg):
    """Per-core degree-descending node ranking and per-block padded length."""
    deg2 = deg.reshape(NCORES, RS)
    rank_order = np.argsort(-deg2, axis=1, kind="stable").astype(np.int32)
    degsorted = np.take_along_axis(deg2, rank_order, axis=1)
    dpad = np.zeros((NCORES, NPAD), np.int32)
    dpad[:, :RS] = degsorted
    Lb = dpad.reshape(NCORES, BPC, 128).max(axis=2).max(axis=0)
    Lb = np.maximum(((Lb + QL - 1) // QL) * QL, QL).astype(np.int64)

    # Coalesce short class runs (except a trailing one) into the previous,
    # larger L: a few extra zero-padded slots buy fewer, bigger pieces, so
    # the per-piece semaphore-latency chains at the pipeline tail collapse.
    start = 0
    n = len(Lb)
    while start < n:
        L = Lb[start]
        e = start
        while e < n and Lb[e] == L:
            e += 1
        if e - start < MINB and e < n:
            upto = min(start + MINB, n)
            Lb[start:upto] = L
        else:
            start = e
    return rank_order, Lb


def _prep(phase, K, edge_index):
    """Host layout: dst-bucketed degree-padded streams + node permutation."""
    ei = np.asarray(edge_index)
    row = ei[0].astype(np.int32)
    col = ei[1].astype(np.int32)
    dst = np.concatenate([row, col])
    src = np.concatenate([col, row])
    w = np.concatenate([K, K]).astype(np.float32)

    order = np.argsort(dst, kind="stable")
    dsts = dst[order]
    srcs = src[order]
    ws = w[order]
    deg = np.bincount(dsts, minlength=N).astype(np.int32)
    starts = np.concatenate([[0], np.cumsum(deg)]).astype(np.int64)
    occ = (np.arange(dsts.size, dtype=np.int64) - starts[dsts]).astype(np.int32)

    delta = phase[srcs] - phase[dsts]
    delta = np.mod(delta + PI, TWO_PI) - PI

    rank_order, Lb = _blocks(deg)
    pieces, TOTW, colbase = _schedule(Lb)

    rank_of = np.empty((NCORES, RS), np.int32)
    np.put_along_axis(rank_of, rank_order,
                      np.broadcast_to(np.arange(RS, dtype=np.int32), (NCORES, RS)),
                      axis=1)

    core_e = dsts // RS
    dloc_e = dsts - core_e * RS
    rank_e = rank_of[core_e, dloc_e].astype(np.int64)
    p_e = rank_e % 128
    c_e = rank_e // 128
    flat = (core_e.astype(np.int64) * 128 + p_e) * TOTW + colbase[c_e] + occ

    ph_str = np.zeros(NCORES * 128 * TOTW, np.float32)
    w_str = np.zeros(NCORES * 128 * TOTW, np.float32)
    ph_str[flat] = delta
    w_str[flat] = ws
    ph_str = ph_str.reshape(NCORES, 128, TOTW)
    w_str = w_str.reshape(NCORES, 128, TOTW)
    return pieces, TOTW, ph_str, w_str, rank_order


def kernel(phase, dphase, power, mass, gamma, K, edge_index):
    phase = np.asarray(phase, np.float32)
    dphase = np.asarray(dphase, np.float32)
    power = np.asarray(power, np.float32)
    mass = np.asarray(mass, np.float32)
    gamma = np.asarray(gamma, np.float32)
    K = np.asarray(K, np.float32)

    pieces, TOTW, ph_str, w_str, rank_order = _prep(phase, K, edge_index)
    key = (TOTW, tuple(pieces))
    if key not in _CACHE:
        _CACHE[key] = _build(pieces, TOTW)
    nc = _CACHE[key]

    in_maps = []
    for ci in range(NCORES):
        nodes = np.zeros((4, NPAD), np.float32)
        nodes[3, RS:] = 1.0                      # dummy-rank mass
        ro = rank_order[ci]
        sl = slice(ci * RS, (ci + 1) * RS)
        nodes[0, :RS] = power[sl][ro]
        nodes[1, :RS] = gamma[sl][ro]
        nodes[2, :RS] = dphase[sl][ro]
        nodes[3, :RS] = mass[sl][ro]
        # rank r = 128*c + p  ->  [128, RT] at (p, c)
        nodes4 = np.ascontiguousarray(
            nodes.reshape(4, RT, 128).transpose(0, 2, 1))
        in_maps.append({"ph": ph_str[ci], "w": w_str[ci], "nodes": nodes4})

    res = run_bass_kernel_spmd(nc, in_maps, core_ids=list(range(NCORES)))
    out = np.empty(N, np.float32)
    for ci in range(NCORES):
        o = res.results[ci]["out"]               # [128, RT], rank = 128*c + p
        by_rank = o.T.reshape(-1)[:RS]
        out[ci * RS + rank_order[ci]] = by_rank
    return out
